# revision 3
# baseline (speedup 1.0000x reference)
"""Multi-head self-attention (RoPE, causal) on 8 Trainium2 NeuronCores.

Sharding: tensor-parallel over heads. Each core owns 2 of 16 heads:
  - QKV projections column-sharded (each core computes its 128 features)
  - attention per (batch, head) pair fully on-core, scores kept in the
    transposed orientation [tk, tq] so softmax needs no PE transposes:
    exp on ACT, denominator via a ones-row appended to V, causal handled
    block-wise + a triangular mask on diagonal blocks
  - AllToAll switches from head-sharding to token-sharding (4MB/core)
  - output projection token-sharded, output written in natural layout

dtypes: fp32r (TF32-like, full PE rate at N>=512) everywhere except the
softmax probabilities / V operand of the PV matmul, which are bf16.
"""

import numpy as np
import ml_dtypes

import concourse.bacc as bacc
import concourse.mybir as mybir
import concourse.tile as tile
from concourse import bass_utils
from concourse.masks import make_identity

F32 = mybir.dt.float32
F32R = mybir.dt.float32r
BF16 = mybir.dt.bfloat16

B, T, D = 4, 2048, 1024
H, DH = 16, 64
N_CORES = 8
HPC = H // N_CORES            # heads per core = 2
EC = HPC * DH                 # feature slice per core = 128
NT = B * T                    # 8192 tokens
TPC = NT // N_CORES           # 1024 tokens per core
THETA = 10000.0

_CACHE = {}
last_results = None  # BassKernelResults of the most recent run (for profiling)


def _build_program():
    nc = bacc.Bacc("TRN2", debug=False, target_bir_lowering=False,
                   num_devices=N_CORES)

    xt_d = nc.dram_tensor("xt", [128, 8, NT], F32R, kind="ExternalInput")
    wq_d = nc.dram_tensor("wq", [128, 8, EC], F32R, kind="ExternalInput")
    wk_d = nc.dram_tensor("wk", [128, 8, EC], F32R, kind="ExternalInput")
    wv_d = nc.dram_tensor("wv", [128, 8, EC], F32R, kind="ExternalInput")
    wo_d = nc.dram_tensor("wo", [128, 8, D], F32R, kind="ExternalInput")
    cos_d = nc.dram_tensor("cosb", [128, T], F32, kind="ExternalInput")
    sin_d = nc.dram_tensor("sinb", [128, T], F32, kind="ExternalInput")
    rotm_d = nc.dram_tensor("rotm", [128, 128], F32R, kind="ExternalInput")
    tri_d = nc.dram_tensor("trimask", [128, 128], BF16, kind="ExternalInput")
    y_d = nc.dram_tensor("y", [TPC, D], F32, kind="ExternalOutput")

    NB = T // 128      # 16 tk blocks per batch
    NCHUNK = NT // 512  # 16 phase-1 chunks

    with tile.TileContext(nc) as tc:
        with (
            tc.tile_pool(name="consts", bufs=1) as consts,
            tc.tile_pool(name="wpool", bufs=1) as wpool,
            tc.tile_pool(name="big", bufs=1) as big,
            tc.tile_pool(name="xp", bufs=2) as xp,
            tc.tile_pool(name="stage", bufs=2) as stage,
            tc.tile_pool(name="expp", bufs=2) as expp,
            tc.tile_pool(name="outp", bufs=2) as outp,
            tc.tile_pool(name="pp", bufs=2, space="PSUM") as pp,
            tc.tile_pool(name="rp", bufs=2, space="PSUM") as rp,
            tc.tile_pool(name="sp", bufs=1, space="PSUM") as sp,
            tc.tile_pool(name="pvp", bufs=2, space="PSUM") as pvp,
            tc.tile_pool(name="dram", bufs=1, space="DRAM") as dram,
        ):
            # ---- constants ----
            cos_sb = consts.tile([128, T], F32)
            sin_sb = consts.tile([128, T], F32)
            rotm_sb = consts.tile([128, 128], F32R)
            tri_sb = consts.tile([128, 128], BF16)
            ident_sb = consts.tile([128, 128], F32)
            nc.sync.dma_start(cos_sb[:], cos_d[:, :])
            nc.sync.dma_start(sin_sb[:], sin_d[:, :])
            nc.sync.dma_start(rotm_sb[:], rotm_d[:, :])
            nc.sync.dma_start(tri_sb[:], tri_d[:, :])
            make_identity(nc, ident_sb[:])

            wq_sb = consts.tile([128, 8, EC], F32R)
            wk_sb = consts.tile([128, 8, EC], F32R)
            wv_sb = consts.tile([128, 8, EC], F32R)
            nc.sync.dma_start(wq_sb[:], wq_d[:, :, :])
            nc.sync.dma_start(wk_sb[:], wk_d[:, :, :])
            nc.sync.dma_start(wv_sb[:], wv_d[:, :, :])

            # ---- persistent tensors ----
            qT = big.tile([128, NT], F32R, tag="qT")
            kT = big.tile([128, NT], F32R, tag="kT")
            # V per (pair, tk-block): [tk=128, 65] with ones in col 64
            vext = big.tile([128, HPC * B, NB, 65], BF16, tag="vext")
            nc.vector.memset(vext[:, :, :, 64], 1.0)

            a2a_in = dram.tile([N_CORES, 128, TPC], F32R)
            a2a_out = dram.tile([N_CORES, 128, TPC], F32R)

            # ================= Phase 1: QKV projections + RoPE =============
            for ci in range(NCHUNK):
                t0 = 512 * ci
                bb = t0 // T
                s0 = t0 % T
                xt = xp.tile([128, 8, 512], F32R, tag="x")
                nc.sync.dma_start(xt[:], xt_d[:, :, t0:t0 + 512])

                for which in ("q", "k", "v"):
                    w_sb = {"q": wq_sb, "k": wk_sb, "v": wv_sb}[which]
                    pt = pp.tile([128, 512], F32, tag="proj")
                    for ko in range(8):
                        nc.tensor.matmul(pt[:], w_sb[:, ko, :], xt[:, ko, :],
                                         start=(ko == 0), stop=(ko == 7))
                    if which in ("q", "k"):
                        raw = stage.tile([128, 512], F32R, tag="raw" + which)
                        nc.scalar.copy(raw[:], pt[:])
                        rot = rp.tile([128, 512], F32, tag="rot")
                        nc.tensor.matmul(rot[:], rotm_sb[:], raw[:],
                                         start=True, stop=True)
                        t1 = stage.tile([128, 512], F32, tag="t1")
                        nc.vector.tensor_tensor(
                            t1[:], raw[:], cos_sb[:, s0:s0 + 512],
                            mybir.AluOpType.mult)
                        t2 = stage.tile([128, 512], F32, tag="t2")
                        nc.vector.tensor_tensor(
                            t2[:], rot[:], sin_sb[:, s0:s0 + 512],
                            mybir.AluOpType.mult)
                        dest = qT if which == "q" else kT
                        nc.vector.tensor_tensor(
                            dest[:, t0:t0 + 512], t1[:], t2[:],
                            mybir.AluOpType.add)
                    else:
                        vraw = stage.tile([128, 512], F32, tag="vraw")
                        nc.scalar.copy(vraw[:], pt[:])
                        for h in range(HPC):
                            pair = bb * HPC + h
                            for bi in range(4):
                                jg = s0 // 128 + bi
                                tp = rp.tile([128, 64], F32, tag="rot",
                                             name="vtr")
                                nc.tensor.transpose(
                                    tp[:],
                                    vraw[64 * h:64 * h + 64,
                                         128 * bi:128 * bi + 128],
                                    ident_sb[64 * h:64 * h + 64,
                                             64 * h:64 * h + 64])
                                nc.vector.tensor_copy(
                                    vext[:, pair, jg, 0:64], tp[:])

            # ================= Phase 2: attention ==========================
            for pair in range(B * HPC):
                bb, h = divmod(pair, HPC)
                tb0 = bb * T
                qh = qT[64 * h:64 * h + 64, tb0:tb0 + T]
                kh = kT[64 * h:64 * h + 64, tb0:tb0 + T]
                for c2 in range(2):
                    jmax = 8 * (c2 + 1)
                    pvt = [pvp.tile([65, 512], F32, tag="pv", name=f"pv_{pair}_{c2}_{hf}")
                           for hf in range(2)]
                    for j in range(jmax):
                        spt = sp.tile([128, 1024], F32, tag="s")
                        lo = max(0, 128 * j - 1024 * c2)
                        for hf in range(2):
                            cl0 = 1024 * c2 + 512 * hf
                            if cl0 + 512 > 128 * j:
                                nc.tensor.matmul(
                                    spt[:, 512 * hf:512 * hf + 512],
                                    kh[:, 128 * j:128 * j + 128],
                                    qh[:, cl0:cl0 + 512],
                                    start=True, stop=True)
                        ex = expp.tile([128, 1024], BF16, tag="e")
                        nc.scalar.activation(
                            ex[:, lo:1024], spt[:, lo:1024],
                            mybir.ActivationFunctionType.Exp, scale=0.125)
                        if 128 * j >= 1024 * c2:
                            nc.vector.tensor_tensor(
                                ex[:, lo:lo + 128], ex[:, lo:lo + 128],
                                tri_sb[:], mybir.AluOpType.mult)
                        vt = vext[:, pair, j, :]  # j-th tk block of this batch
                        for hf in range(2):
                            h0 = 512 * hf
                            a = max(h0, lo)
                            if a < h0 + 512:
                                last_j = min(jmax - 1, 8 * c2 + 4 * hf + 3)
                                nc.tensor.matmul(
                                    pvt[hf][:, a - h0:512],
                                    vt, ex[:, a:h0 + 512],
                                    start=(j == 0), stop=(j == last_j))
                    # normalize + ship to a2a_in
                    for hf in range(2):
                        rec = outp.tile([1, 512], F32, tag="rec")
                        nc.vector.reciprocal(rec[:], pvt[hf][64:65, :])
                        recb = outp.tile([64, 512], F32, tag="recb")
                        nc.gpsimd.partition_broadcast(recb[:], rec[:])
                        ao = outp.tile([64, 512], F32R, tag="ao")
                        nc.vector.tensor_tensor(
                            ao[:], pvt[hf][0:64, :], recb[:],
                            mybir.AluOpType.mult)
                        dcore = 2 * bb + c2
                        nc.sync.dma_start(
                            a2a_in[dcore, 64 * h:64 * h + 64,
                                   512 * hf:512 * hf + 512],
                            ao[:])

            # ================= Phase 3: A2A + output projection ============
            nc.gpsimd.collective_compute(
                "AllToAll", mybir.AluOpType.bypass,
                replica_groups=[list(range(N_CORES))],
                ins=[a2a_in.opt()], outs=[a2a_out.opt()])

            oall = big.tile([128, 8, TPC], F32R, tag="qT")
            nc.sync.dma_start(oall[:], a2a_out[:].rearrange("s p t -> p s t"))
            for eo in range(2):
                wo_sb = wpool.tile([128, 8, 512], F32R, tag="wo")
                nc.sync.dma_start(wo_sb[:], wo_d[:, :, 512 * eo:512 * eo + 512])
                for tb in range(TPC // 128):
                    ot = pp.tile([128, 512], F32, tag="proj")
                    for ec in range(8):
                        nc.tensor.matmul(
                            ot[:], oall[:, ec, 128 * tb:128 * tb + 128],
                            wo_sb[:, ec, :],
                            start=(ec == 0), stop=(ec == 7))
                    ys = outp.tile([128, 512], F32, tag="y")
                    nc.scalar.copy(ys[:], ot[:])
                    nc.sync.dma_start(
                        y_d[128 * tb:128 * tb + 128,
                            512 * eo:512 * eo + 512], ys[:])

    nc.compile()
    return nc


def _host_inputs(x, Wq, Wk, Wv, Wo, token_positions):
    """Per-core in_maps with transposed/tiled layouts."""
    x = np.asarray(x, dtype=np.float32)
    xt = np.ascontiguousarray(
        x.reshape(NT, D).T.reshape(8, 128, NT).transpose(1, 0, 2))

    pos = np.asarray(token_positions).astype(np.float64)
    inv_freq = 1.0 / (THETA ** (np.arange(0, DH, 2, dtype=np.float64) / DH))
    ang = pos[None, :] * inv_freq[:, None]          # [32, T]
    cos_p = np.cos(ang)                              # pair i
    sin_p = np.sin(ang)
    # partition p (0..127): within-head dim d = p % 64, pair = d // 2
    d_idx = (np.arange(128) % 64) // 2
    cosb = cos_p[d_idx, :].astype(np.float32)
    sinb = sin_p[d_idx, :].astype(np.float32)

    rotm = np.zeros((128, 128), dtype=np.float32)
    for i in range(64):
        rotm[2 * i + 1, 2 * i] = -1.0   # out[2i] -= in[2i+1]*sin -> rot[2i] = -in[2i+1]
        rotm[2 * i, 2 * i + 1] = 1.0    # rot[2i+1] = in[2i]
    tri = np.tril(np.ones((128, 128), dtype=np.float32)).T  # [tk, tq] tk<=tq
    tri = tri.astype(ml_dtypes.bfloat16)

    def wtiles(W, sl):
        # lhsT tiles: [p, ko, e] with d = ko*128+p contracting
        Wt = np.ascontiguousarray(W[sl, :].T)        # [D, e]
        return np.ascontiguousarray(
            Wt.reshape(8, 128, Wt.shape[1]).transpose(1, 0, 2))

    WoT = np.ascontiguousarray(np.asarray(Wo, dtype=np.float32).T)  # [e_in, e_out]
    wo_t = np.ascontiguousarray(WoT.reshape(8, 128, D).transpose(1, 0, 2))

    in_maps = []
    for c in range(N_CORES):
        sl = slice(EC * c, EC * (c + 1))
        in_maps.append({
            "xt": xt,
            "wq": wtiles(np.asarray(Wq, np.float32), sl),
            "wk": wtiles(np.asarray(Wk, np.float32), sl),
            "wv": wtiles(np.asarray(Wv, np.float32), sl),
            "wo": wo_t,
            "cosb": cosb,
            "sinb": sinb,
            "rotm": rotm,
            "trimask": tri,
        })
    return in_maps


def kernel(x, Wq, Wk, Wv, Wo, token_positions):
    global last_results
    if "nc" not in _CACHE:
        _CACHE["nc"] = _build_program()
    nc = _CACHE["nc"]
    in_maps = _host_inputs(x, Wq, Wk, Wv, Wo, token_positions)
    res = bass_utils.run_bass_kernel_spmd(nc, in_maps, list(range(N_CORES)))
    last_results = res
    y = np.concatenate([res.results[c]["y"] for c in range(N_CORES)], axis=0)
    return y.reshape(B, T, D)


# revision 23
# speedup vs baseline: 1.4187x; 1.4187x over previous
"""Multi-head self-attention (RoPE, causal) on 8 Trainium2 NeuronCores.

Sharding: tensor-parallel over heads. Each core owns 2 of 16 heads:
  - QKV projections column-sharded (each core computes its 128 features)
  - attention per (batch, head) pair fully on-core, scores kept in the
    transposed orientation [tk, tq] so softmax needs no PE transposes:
    exp on ACT, denominator via a ones-row appended to V, causal handled
    block-wise + a triangular mask on diagonal blocks
  - AllToAll switches from head-sharding to token-sharding (4MB/core)
  - output projection token-sharded, output written in natural layout

dtypes: fp32r (TF32-like, full PE rate at N>=512) everywhere except the
softmax probabilities / V operand of the PV matmul, which are bf16.
"""

import numpy as np
import ml_dtypes

import concourse.bacc as bacc
import concourse.mybir as mybir
import concourse.tile as tile
from concourse import bass_utils
from concourse.masks import make_identity

F32 = mybir.dt.float32
F32R = mybir.dt.float32r
BF16 = mybir.dt.bfloat16

B, T, D = 4, 2048, 1024
H, DH = 16, 64
N_CORES = 8
HPC = H // N_CORES            # heads per core = 2
EC = HPC * DH                 # feature slice per core = 128
NT = B * T                    # 8192 tokens
TPC = NT // N_CORES           # 1024 tokens per core
THETA = 10000.0

_CACHE = {}
last_results = None  # BassKernelResults of the most recent run (for profiling)


def _build_program():
    nc = bacc.Bacc("TRN2", debug=False, target_bir_lowering=False,
                   num_devices=N_CORES)

    xt_d = nc.dram_tensor("xt", [128, 8, NT], BF16, kind="ExternalInput")
    wq_d = nc.dram_tensor("wq", [128, 8, EC], BF16, kind="ExternalInput")
    wk_d = nc.dram_tensor("wk", [128, 8, EC], BF16, kind="ExternalInput")
    wv_d = nc.dram_tensor("wv", [128, 8, EC], BF16, kind="ExternalInput")
    wo_d = nc.dram_tensor("wo", [128, 8, D], BF16, kind="ExternalInput")
    cos_d = nc.dram_tensor("cosb", [128, T], F32, kind="ExternalInput")
    sin_d = nc.dram_tensor("sinb", [128, T], F32, kind="ExternalInput")
    rotm_d = nc.dram_tensor("rotm", [128, 128], F32R, kind="ExternalInput")
    tri_d = nc.dram_tensor("trimask", [128, 128], BF16, kind="ExternalInput")
    y_d = nc.dram_tensor("y", [TPC, D], F32, kind="ExternalOutput")

    NB = T // 128      # 16 tk blocks per batch
    NCHUNK = NT // 512  # 16 phase-1 chunks

    with tile.TileContext(nc) as tc:
        with (
            tc.tile_pool(name="consts", bufs=1) as consts,
            tc.tile_pool(name="wpool", bufs=1) as wpool,
            tc.tile_pool(name="big", bufs=1) as big,
            tc.tile_pool(name="xp", bufs=2) as xp,
            tc.tile_pool(name="stage", bufs=2) as stage,
            tc.tile_pool(name="expp", bufs=4) as expp,
            tc.tile_pool(name="outp", bufs=2) as outp,
            tc.tile_pool(name="psA", bufs=1, space="PSUM") as psA,
            tc.tile_pool(name="psB", bufs=1, space="PSUM") as psB,
            tc.tile_pool(name="pvA", bufs=2, space="PSUM") as pvA,
            tc.tile_pool(name="pvB", bufs=2, space="PSUM") as pvB,
            tc.tile_pool(name="dram", bufs=2, space="DRAM") as dram,
        ):
            # ---- constants ----
            cos_sb = consts.tile([128, T], F32)
            sin_sb = consts.tile([128, T], F32)
            rotm_sb = consts.tile([128, 128], F32R)
            tri_sb = consts.tile([128, 128], BF16)
            ident_sb = consts.tile([128, 128], F32)
            nc.sync.dma_start(cos_sb[:], cos_d[:, :])
            nc.sync.dma_start(sin_sb[:], sin_d[:, :])
            nc.sync.dma_start(rotm_sb[:], rotm_d[:, :])
            nc.sync.dma_start(tri_sb[:], tri_d[:, :])
            make_identity(nc, ident_sb[:])

            wq_sb = consts.tile([128, 8, EC], BF16)
            wk_sb = consts.tile([128, 8, EC], BF16)
            wv_sb = consts.tile([128, 8, EC], BF16)
            nc.sync.dma_start(wq_sb[:], wq_d[:, :, :])
            nc.sync.dma_start(wk_sb[:], wk_d[:, :, :])
            nc.sync.dma_start(wv_sb[:], wv_d[:, :, :])

            # ---- persistent tensors ----
            qT = big.tile([128, NT], F32R, tag="qT")
            kT = big.tile([128, NT], F32R, tag="kT")
            # V per (pair, tk-block): [tk=128, 65] with ones in col 64
            vext = big.tile([128, HPC * B, NB, 65], BF16, tag="vext")
            nc.vector.memset(vext[:, :, :, 64], 1.0)

            a2a_in1 = dram.tile([N_CORES, 128, 768], BF16)
            a2a_out1 = dram.tile([N_CORES, 128, 768], BF16)
            a2a_in2 = dram.tile([N_CORES, 128, 256], BF16)
            a2a_out2 = dram.tile([N_CORES, 128, 256], BF16)

            # ================= Phase 1: QKV projections + RoPE =============
            def do_chunk(ci):
                t0 = 512 * ci
                bb = t0 // T
                s0 = t0 % T
                xt = xp.tile([128, 8, 512], BF16, tag="x")
                nc.sync.dma_start(xt[:], xt_d[:, :, t0:t0 + 512])

                # pipelined: proj(q) -> ACTcopy(q) -> proj(k) -> rot(q) ->
                # ACTcopy(k) -> proj(v) -> rot(k) -> ACTcopy(v) -> vtrans
                # so PE never sits behind an ACT drain.
                def _proj(w_sb, nm, pool):
                    pt = pool.tile([128, 1024], F32, tag="s", name="p" + nm)
                    pt = pt[:, 0:512]
                    for ko in range(8):
                        nc.tensor.matmul(pt, w_sb[:, ko, :], xt[:, ko, :],
                                         start=(ko == 0), stop=(ko == 7))
                    return pt

                def _rot(raw, nm, pool):
                    rot = pool.tile([128, 1024], F32, tag="s", name="r" + nm)
                    rot = rot[:, 0:512]
                    nc.tensor.matmul(rot, rotm_sb[:], raw[:],
                                     start=True, stop=True)
                    return rot

                def _rope_combine(raw, rot, dest):
                    t1 = stage.tile([128, 512], F32, tag="t1")
                    nc.vector.tensor_tensor(
                        t1[:], raw[:], cos_sb[:, s0:s0 + 512],
                        mybir.AluOpType.mult)
                    t2 = stage.tile([128, 512], F32, tag="t2")
                    nc.vector.tensor_tensor(
                        t2[:], rot[:], sin_sb[:, s0:s0 + 512],
                        mybir.AluOpType.mult)
                    nc.vector.tensor_tensor(
                        dest[:, t0:t0 + 512], t1[:], t2[:],
                        mybir.AluOpType.add)

                pq = _proj(wq_sb, "q", psA)
                rawq = stage.tile([128, 512], F32R, tag="rawq")
                nc.scalar.copy(rawq[:], pq)
                pk = _proj(wk_sb, "k", psB)
                rotq = _rot(rawq, "q", psA)
                rawk = stage.tile([128, 512], F32R, tag="rawk")
                nc.scalar.copy(rawk[:], pk)
                pv_ = _proj(wv_sb, "v", psB)
                rotk = _rot(rawk, "k", psA)
                vraw = stage.tile([128, 512], F32, tag="vraw")
                nc.scalar.copy(vraw[:], pv_)
                _rope_combine(rawq, rotq, qT)
                _rope_combine(rawk, rotk, kT)
                for h in range(HPC):
                    pair = bb * HPC + h
                    for bi in range(4):
                        jg = s0 // 128 + bi
                        tp = (psB if bi % 2 else psA).tile(
                            [128, 1024], F32, tag="s", name="vtr")[:, 0:64]
                        nc.tensor.transpose(
                            tp,
                            vraw[64 * h:64 * h + 64,
                                 128 * bi:128 * bi + 128],
                            ident_sb[64 * h:64 * h + 64,
                                     64 * h:64 * h + 64])
                        nc.vector.tensor_copy(
                            vext[:, pair, jg, 0:64], tp)

            # ================= Phase 2: attention ==========================
            # Two heads of the same batch run as interleaved pipeline
            # streams: ACT-exp latency of one stream hides behind PE work
            # of the other.
            def do_attn(bb):
                tb0 = bb * T
                qs = [qT[64 * hh:64 * hh + 64, tb0:tb0 + T] for hh in range(2)]
                ks = [kT[64 * hh:64 * hh + 64, tb0:tb0 + T] for hh in range(2)]
                spools = [psA, psB]
                vpools = [pvA, pvB]
                for c2 in range(2):
                    jmax = 8 * (c2 + 1)
                    pvt = [[vpools[hh].tile(
                        [65, 512], F32, tag="pv",
                        name=f"pv_{bb}_{hh}_{c2}_{hf}") for hf in range(2)]
                        for hh in range(2)]

                    def _scores_pair(j):
                        # both heads' score matmuls, issued alternating so
                        # the two K=64 row-strips (partitions 0-63 / 64-127)
                        # execute concurrently in the PE array.
                        spts = [spools[hh].tile(
                            [128, 1024], F32, tag="s",
                            name=f"s_{bb}_{hh}_{c2}_{j}") for hh in range(2)]
                        for hf in range(2):
                            cl0 = 1024 * c2 + 512 * hf
                            if cl0 + 512 <= 128 * j:
                                continue
                            w = cl0 + 512 - max(cl0, 128 * j)
                            N = 512 if w == 512 else max(256, w)
                            st = cl0 + 512 - N
                            for hh in range(2):
                                nc.tensor.matmul(
                                    spts[hh][:, st - 1024 * c2:
                                             st - 1024 * c2 + N],
                                    ks[hh][:, 128 * j:128 * j + 128],
                                    qs[hh][:, st:st + N],
                                    start=True, stop=True)
                        return spts

                    def _pv(j, exs):
                        lo = max(0, 128 * j - 1024 * c2)
                        for hh in range(2):
                            vt = vext[:, bb * HPC + hh, j, :]
                            for hf in range(2):
                                h0 = 512 * hf
                                a = max(h0, lo)
                                if a < h0 + 512:
                                    last_j = min(jmax - 1,
                                                 8 * c2 + 4 * hf + 3)
                                    nc.tensor.matmul(
                                        pvt[hh][hf][:, a - h0:512],
                                        vt, exs[hh][:, a:h0 + 512],
                                        start=(j == 0), stop=(j == last_j))

                    # software pipeline with one-iteration PV delay: the PE
                    # always has ready work (PV of j-1) at its queue head
                    # while ACT computes exp(j).
                    spt = _scores_pair(0)
                    prev = None
                    for j in range(jmax):
                        lo = max(0, 128 * j - 1024 * c2)
                        exs = []
                        for hh in range(2):
                            ex = expp.tile([128, 1024], BF16, tag="e",
                                           name=f"e_{hh}")
                            nc.scalar.activation(
                                ex[:, lo:1024], spt[hh][:, lo:1024],
                                mybir.ActivationFunctionType.Exp, scale=0.125)
                            exs.append(ex)
                        if prev is not None:
                            _pv(prev[0], prev[1])
                        if j + 1 < jmax:
                            spt = _scores_pair(j + 1)
                        for hh in range(2):
                            if 128 * j >= 1024 * c2:
                                nc.vector.tensor_tensor(
                                    exs[hh][:, lo:lo + 128],
                                    exs[hh][:, lo:lo + 128],
                                    tri_sb[:], mybir.AluOpType.mult)
                        prev = (j, exs)
                    _pv(prev[0], prev[1])
                    # normalize + ship to a2a_in.  Copy psum out first
                    # (ACT) so the pv slots free up for the next chunk.
                    for hh in range(2):
                        dnm = outp.tile([33, 512], F32, tag="dnm")
                        unn = [None, None]
                        for hf in range(2):
                            nc.vector.tensor_copy(
                                dnm[32 * hf:32 * hf + 1, :],
                                pvt[hh][hf][64:65, :])
                            unn[hf] = outp.tile([64, 512], BF16,
                                                tag=f"unn{hf}",
                                                name=f"unn{hf}")
                            nc.scalar.copy(unn[hf][:], pvt[hh][hf][0:64, :])
                        rec = outp.tile([33, 512], F32, tag="rec")
                        nc.vector.reciprocal(rec[:], dnm[:])
                        rscr = dram.tile([2, 512], F32, tag="rscr",
                                         name="rscr")
                        for hf in range(2):
                            nc.sync.dma_start(rscr[hf:hf + 1, :],
                                              rec[32 * hf:32 * hf + 1, :])
                        for hf in range(2):
                            recb = outp.tile([64, 512], F32, tag="recb")
                            nc.sync.dma_start(
                                recb[:],
                                rscr[hf:hf + 1, :].to_broadcast((64, 512)))
                            ao = outp.tile([64, 512], BF16, tag="ao")
                            nc.vector.tensor_tensor(
                                ao[:], unn[hf][:], recb[:],
                                mybir.AluOpType.mult)
                            # group 1 = batches 0-2 (768 tokens/dest),
                            # group 2 = batch 3 (256 tokens/dest)
                            if bb < 3:
                                grp, base, W = a2a_in1, 0, 768
                            else:
                                grp, base, W = a2a_in2, 6144, 256
                            tt = 2048 * bb + 1024 * c2 + 512 * hf - base
                            off = 0
                            while off < 512:
                                dd = (tt + off) // W
                                col = (tt + off) % W
                                w = min(512 - off, W - col)
                                nc.sync.dma_start(
                                    grp[dd, 64 * hh:64 * hh + 64,
                                        col:col + w],
                                    ao[:, off:off + w])
                                off += w

            def do_oproj(g, oall_g, row0, ntb):
                # y rows [row0, row0 + 128*ntb) from group-g tokens
                for eo in range(2):
                    wo_sb = wpool.tile([128, 8, 512], BF16, tag="wo",
                                       name=f"wo_{g}_{eo}")
                    nc.sync.dma_start(wo_sb[:],
                                      wo_d[:, :, 512 * eo:512 * eo + 512])
                    for tb in range(ntb):
                        ot = (psB if (tb + eo) % 2 else psA).tile(
                            [128, 1024], F32, tag="s", name="ot")[:, 0:512]
                        for ec in range(8):
                            nc.tensor.matmul(
                                ot, oall_g[:, ec, 128 * tb:128 * tb + 128],
                                wo_sb[:, ec, :],
                                start=(ec == 0), stop=(ec == 7))
                        ys = outp.tile([128, 512], F32, tag="y")
                        nc.scalar.copy(ys[:], ot)
                        nc.sync.dma_start(
                            y_d[row0 + 128 * tb:row0 + 128 * tb + 128,
                                512 * eo:512 * eo + 512], ys[:])

            # interleave phase 1 and attention per batch; group-0 A2A and
            # its output projection overlap batches 2-3.
            rg = [list(range(N_CORES))]
            for bb in range(3):
                for ci in range(4 * bb, 4 * bb + 4):
                    do_chunk(ci)
                do_attn(bb)
            nc.gpsimd.collective_compute(
                "AllToAll", mybir.AluOpType.bypass, replica_groups=rg,
                ins=[a2a_in1.opt()], outs=[a2a_out1.opt()])
            oall1 = wpool.tile([128, 8, 768], BF16, tag="oall1")
            nc.sync.dma_start(oall1[:],
                              a2a_out1[:].rearrange("s p t -> p s t"))
            for ci in range(12, 16):
                do_chunk(ci)
            do_attn(3)
            nc.gpsimd.collective_compute(
                "AllToAll", mybir.AluOpType.bypass, replica_groups=rg,
                ins=[a2a_in2.opt()], outs=[a2a_out2.opt()])
            do_oproj(0, oall1, 0, 6)
            oall2 = wpool.tile([128, 8, 256], BF16, tag="oall2")
            nc.sync.dma_start(oall2[:],
                              a2a_out2[:].rearrange("s p t -> p s t"))
            do_oproj(1, oall2, 768, 2)

    nc.compile()
    return nc


def _host_inputs(x, Wq, Wk, Wv, Wo, token_positions):
    """Per-core in_maps with transposed/tiled layouts."""
    x = np.asarray(x, dtype=np.float32)
    xt_bf = np.ascontiguousarray(
        x.reshape(NT, D).T.reshape(8, 128, NT).transpose(1, 0, 2)
    ).astype(ml_dtypes.bfloat16)

    pos = np.asarray(token_positions).astype(np.float64)
    inv_freq = 1.0 / (THETA ** (np.arange(0, DH, 2, dtype=np.float64) / DH))
    ang = pos[None, :] * inv_freq[:, None]          # [32, T]
    cos_p = np.cos(ang)                              # pair i
    sin_p = np.sin(ang)
    # partition p (0..127): within-head dim d = p % 64, pair = d // 2
    d_idx = (np.arange(128) % 64) // 2
    cosb = cos_p[d_idx, :].astype(np.float32)
    sinb = sin_p[d_idx, :].astype(np.float32)

    rotm = np.zeros((128, 128), dtype=np.float32)
    for i in range(64):
        rotm[2 * i + 1, 2 * i] = -1.0   # out[2i] -= in[2i+1]*sin -> rot[2i] = -in[2i+1]
        rotm[2 * i, 2 * i + 1] = 1.0    # rot[2i+1] = in[2i]
    tri = np.tril(np.ones((128, 128), dtype=np.float32)).T  # [tk, tq] tk<=tq
    tri = tri.astype(ml_dtypes.bfloat16)

    def wtiles(W, sl):
        # lhsT tiles: [p, ko, e] with d = ko*128+p contracting
        Wt = np.ascontiguousarray(W[sl, :].T)        # [D, e]
        return np.ascontiguousarray(
            Wt.reshape(8, 128, Wt.shape[1]).transpose(1, 0, 2))

    WoT = np.ascontiguousarray(np.asarray(Wo, dtype=np.float32).T)  # [e_in, e_out]
    wo_t = np.ascontiguousarray(WoT.reshape(8, 128, D).transpose(1, 0, 2))

    in_maps = []
    for c in range(N_CORES):
        sl = slice(EC * c, EC * (c + 1))
        in_maps.append({
            "xt": xt_bf,
            "wq": wtiles(np.asarray(Wq, np.float32), sl).astype(ml_dtypes.bfloat16),
            "wk": wtiles(np.asarray(Wk, np.float32), sl).astype(ml_dtypes.bfloat16),
            "wv": wtiles(np.asarray(Wv, np.float32), sl).astype(ml_dtypes.bfloat16),
            "wo": wo_t.astype(ml_dtypes.bfloat16),
            "cosb": cosb,
            "sinb": sinb,
            "rotm": rotm,
            "trimask": tri,
        })
    return in_maps


def kernel(x, Wq, Wk, Wv, Wo, token_positions):
    global last_results
    if "nc" not in _CACHE:
        _CACHE["nc"] = _build_program()
    nc = _CACHE["nc"]
    in_maps = _host_inputs(x, Wq, Wk, Wv, Wo, token_positions)
    res = bass_utils.run_bass_kernel_spmd(nc, in_maps, list(range(N_CORES)))
    last_results = res
    y = np.empty((NT, D), dtype=np.float32)
    for c in range(N_CORES):
        yc = res.results[c]["y"]
        y[768 * c:768 * c + 768] = yc[0:768]
        y[6144 + 256 * c:6144 + 256 * c + 256] = yc[768:1024]
    return y.reshape(B, T, D)
